# revision 1
# baseline (speedup 1.0000x reference)
"""Trainium2 Bass kernel for YOLO-style DetectionLayer decode.

Full input  x: (16, 255, 76, 76) f32  (channel-major: 3 anchors x 85 ch)
Full output  : (16, 17328, 85) f32   (position-major: 3*76*76 rows x 85 ch)

Math per (b, a, gy, gx):
  out[..., 0] = (sigmoid(tx) + gx) * stride        stride = 8
  out[..., 1] = (sigmoid(ty) + gy) * stride
  out[..., 2] = exp(tw) * ANCHOR[a][0]             (stride cancels)
  out[..., 3] = exp(th) * ANCHOR[a][1]
  out[..., 4:] = sigmoid(...)

Sharding: pure data-parallel over batch: 2 batches per core x 8 cores.

Per-core kernel (per (b, a) pair, 6 pairs):
  - DMA in the contiguous (85, 5776) channel-major block.
  - TensorE transposes 46 chunks of (85 ch, 128 pos) -> PSUM (128 pos, 85 ch).
    Chunk j takes positions {45*p + j} so output partition p holds 45
    consecutive output rows -> the output DMA gets 15.3KB contiguous runs.
  - ScalarE evacuates PSUM with fused tanh(v/2) (sigmoid = .5 + .5*tanh(v/2);
    single ACT table set), plus small Exp ops on w/h cols from PSUM.
  - VectorE fixups: cls/conf = .5*t+.5 ; x/y = 4*t + (8*grid+4) from a host
    table ; w/h *= anchor.
  - One DMA out per pair (plus a tiny 16-row tail).
"""

import os
import sys

import numpy as np

for _p in ("/opt/trn_rl_repo", "/root/.axon_site/_ro/trn_rl_repo"):
    if os.path.isdir(_p) and _p not in sys.path:
        sys.path.append(_p)

import concourse.bacc as bacc
import concourse.bass as bass
import concourse.mybir as mybir
import concourse.tile as tile
from concourse.bass_utils import run_bass_kernel_spmd
from concourse.masks import make_identity

ANCHORS = np.array([[10.0, 13.0], [16.0, 30.0], [33.0, 23.0]], dtype=np.float32)
NB_FULL = 16
N_CORES = 8
NB = NB_FULL // N_CORES  # batches per core
NA = 3
NC = 85  # 5 + 80 channels
NG = 76
NPOS = NG * NG  # 5776
STRIDE = 8.0

# Position-chunking: output partition p holds rows [45p, 45p+45); chunk j
# gathers positions {45p + j}. 5776 = 128*45 + 16 -> 16-row tail.
RPP = 45  # rows per partition (main part)
MAIN = 128 * RPP  # 5760
TAIL = NPOS - MAIN  # 16

F32 = mybir.dt.float32
AF = mybir.ActivationFunctionType
OP = mybir.AluOpType


def _grid_tables():
    """gg[p, 2j+c]: x/y grid offsets (8*grid+4) for row r = 45p+j.
    gxt[t]: x offset for tail rows r = 5760+t (gy is constant 75)."""
    p = np.arange(128)[:, None]
    j = np.arange(RPP)[None, :]
    r = p * RPP + j
    gg = np.empty((128, 2 * RPP), dtype=np.float32)
    gg[:, 0::2] = (r % NG) * STRIDE + 4.0
    gg[:, 1::2] = (r // NG) * STRIDE + 4.0
    rt = MAIN + np.arange(TAIL)
    gxt = ((rt % NG) * STRIDE + 4.0).astype(np.float32)[:, None]
    gyt = float((MAIN // NG) * STRIDE + 4.0)  # rows 5760..5775 all have gy=75
    assert np.all(rt // NG == MAIN // NG)
    return gg, gxt, gyt


GG_TABLE, GXT_TABLE, GYT_CONST = _grid_tables()


def build_program():
    nc = bacc.Bacc(None, target_bir_lowering=False)

    x = nc.dram_tensor("x", (NB, NA * NC, NG, NG), F32, kind="ExternalInput")
    out = nc.dram_tensor("out", (NB, NA * NPOS, NC), F32, kind="ExternalOutput")
    gg = nc.dram_tensor("gg", (128, 2 * RPP), F32, kind="ExternalInput")
    gxt = nc.dram_tensor("gxt", (TAIL, 1), F32, kind="ExternalInput")

    with tile.TileContext(nc) as tc:
        with (
            tc.tile_pool(name="constp", bufs=1) as constp,
            tc.tile_pool(name="xp", bufs=2) as xp,
            tc.tile_pool(name="outp", bufs=2) as outp,
            tc.tile_pool(name="pp", bufs=3, space="PSUM") as pp,
            tc.tile_pool(name="tp", bufs=2, space="PSUM") as tp,
        ):
            ident = constp.tile([128, 128], F32)
            make_identity(nc, ident)
            ggs = constp.tile([128, 2 * RPP], F32)
            nc.sync.dma_start(out=ggs[:], in_=gg[:])
            gxts = constp.tile([TAIL, 1], F32)
            nc.sync.dma_start(out=gxts[:], in_=gxt[:])
            ggv = ggs.rearrange("p (k c) -> p k c", c=2)

            for b in range(NB):
                for a in range(NA):
                    aw = float(ANCHORS[a, 0])
                    ah = float(ANCHORS[a, 1])
                    xt = xp.tile([NC, NPOS], F32, tag="xt")
                    nc.sync.dma_start(
                        out=xt[:],
                        in_=x[b, NC * a : NC * (a + 1)].rearrange("c h w -> c (h w)"),
                    )
                    ot = outp.tile([128, RPP * NC], F32, tag="ot")
                    tt = outp.tile([TAIL, NC], F32, tag="tt")
                    # (85, 45, 128): [:, j, :] = chunk j (stride-45 positions)
                    xmain = xt[:, 0:MAIN].rearrange("c (m j) -> c j m", j=RPP)

                    # main chunks in groups sharing a 2-bank PSUM tile
                    for k0, nk in ((0, 12), (12, 12), (24, 12), (36, 9)):
                        ps = pp.tile([128, 1024], F32, tag="ps")
                        for m in range(nk):
                            off = 512 * (m // 6) + NC * (m % 6)
                            nc.tensor.transpose(
                                ps[:, off : off + NC],
                                xmain[:, k0 + m, :],
                                ident[0:NC, 0:NC],
                            )
                        # evacuate with fused tanh(v/2), bank-major == chunk-major
                        nbk, rem = divmod(nk, 6)
                        if rem == 0:
                            pv = ps.rearrange("p (bk q) -> p bk q", bk=2)[:, :, 0:510]
                            nc.scalar.activation(
                                ot[:, k0 * NC : (k0 + nk) * NC], pv, AF.Tanh, scale=0.5
                            )
                        else:
                            for bk in range(nbk + 1):
                                w = 6 if bk < nbk else rem
                                if w == 0:
                                    continue
                                pv = ps[:, 512 * bk : 512 * bk + w * NC]
                                c0 = (k0 + bk * 6) * NC
                                nc.scalar.activation(
                                    ot[:, c0 : c0 + w * NC], pv, AF.Tanh, scale=0.5
                                )
                        # true exp on the w/h cols, straight from PSUM raw values
                        for bk in range(nbk + 1):
                            w = 6 if bk < nbk else rem
                            if w == 0:
                                continue
                            pwh = (
                                ps[:, 512 * bk : 512 * bk + w * NC]
                                .rearrange("p (k c) -> p k c", c=NC)[:, :, 2:4]
                            )
                            c0 = (k0 + bk * 6) * NC
                            owh = ot[:, c0 : c0 + w * NC].rearrange(
                                "p (k c) -> p k c", c=NC
                            )[:, :, 2:4]
                            nc.scalar.activation(owh, pwh, AF.Exp)

                    # tail: positions 5760..5775
                    pst = tp.tile([TAIL, 512], F32, tag="pst")
                    nc.tensor.transpose(
                        pst[:, 0:NC], xt[:, MAIN:NPOS], ident[0:NC, 0:NC]
                    )
                    nc.scalar.activation(tt[:], pst[:, 0:NC], AF.Tanh, scale=0.5)
                    nc.scalar.activation(tt[:, 2:4], pst[:, 2:4], AF.Exp)

                    # VectorE fixups (main)
                    otr = ot.rearrange("p (k c) -> p k c", c=NC)
                    cls_v = otr[:, :, 4:NC]
                    nc.vector.tensor_scalar(cls_v, cls_v, 0.5, 0.5, OP.mult, OP.add)
                    xy = otr[:, :, 0:2]
                    nc.vector.tensor_scalar(xy, xy, 4.0, None, OP.mult)
                    nc.vector.tensor_tensor(xy, xy, ggv, OP.add)
                    wv = otr[:, :, 2:3]
                    nc.vector.tensor_scalar(wv, wv, aw, None, OP.mult)
                    hv = otr[:, :, 3:4]
                    nc.vector.tensor_scalar(hv, hv, ah, None, OP.mult)

                    # VectorE fixups (tail)
                    nc.vector.tensor_scalar(
                        tt[:, 4:NC], tt[:, 4:NC], 0.5, 0.5, OP.mult, OP.add
                    )
                    nc.vector.tensor_scalar(
                        tt[:, 0:1], tt[:, 0:1], 4.0, gxts[:], OP.mult, OP.add
                    )
                    nc.vector.tensor_scalar(
                        tt[:, 1:2], tt[:, 1:2], 4.0, GYT_CONST, OP.mult, OP.add
                    )
                    nc.vector.tensor_scalar(tt[:, 2:3], tt[:, 2:3], aw, None, OP.mult)
                    nc.vector.tensor_scalar(tt[:, 3:4], tt[:, 3:4], ah, None, OP.mult)

                    # stores
                    base = a * NPOS
                    nc.sync.dma_start(
                        out=out[b, base : base + MAIN, :].rearrange(
                            "(p j) c -> p (j c)", p=128
                        ),
                        in_=ot[:],
                    )
                    nc.sync.dma_start(out=out[b, base + MAIN : base + NPOS, :], in_=tt[:])

    nc.compile()
    return nc


_NC_CACHE = None


def _get_program():
    global _NC_CACHE
    if _NC_CACHE is None:
        _NC_CACHE = build_program()
    return _NC_CACHE


def run(x, trace=False, **kwargs):
    """x: full (16, 255, 76, 76) f32. Returns (full_out, BassKernelResults)."""
    x = np.ascontiguousarray(np.asarray(x, dtype=np.float32))
    assert x.shape == (NB_FULL, NA * NC, NG, NG), x.shape
    nc = _get_program()
    in_maps = [
        {
            "x": np.ascontiguousarray(x[c * NB : (c + 1) * NB]),
            "gg": GG_TABLE,
            "gxt": GXT_TABLE,
        }
        for c in range(N_CORES)
    ]
    res = run_bass_kernel_spmd(nc, in_maps, list(range(N_CORES)), trace=trace, **kwargs)
    out = np.concatenate([res.results[c]["out"] for c in range(N_CORES)], axis=0)
    return out, res


def kernel(x):
    out, _ = run(x, trace=False)
    return out
